# revision 9
# baseline (speedup 1.0000x reference)
"""MoE QKV parallel linear for Trainium2, 8 NeuronCores.

Problem: out[t] = x[t] @ W[id[t]].T with x [16384, 2048] f32,
W [4, 3072, 2048] f32, id sorted int32 (tokens pre-grouped by expert).

Sharding: data-parallel over tokens with expert-pure shards. Since tokens
are sorted by expert, split each expert's contiguous token range across a
proportional share of the 8 cores. Every core runs one dense projection
against a single expert's weight, padded with zero tokens to the common
t_pad so the SPMD program is uniform across cores.

Device kernel (per core), f32 PSUM: W stationary, token columns moving,
output transposed ([qkv, tokens]). Four of the 16 contraction k-tiles
(hidden dims 0..511) run as TWO fp8e4 DoubleRow matmuls (2 MACs/cell/
cycle; quantization error measured 1.84e-2 absmax-rel vs the 2e-2 gate,
exact host emulation); the other 12 run bf16 (Fast-Weight-Load hides
their LDWEIGHTS). The two DoubleRow matmuls are interleaved between bf16
matmuls — a DR LDWEIGHTS (256 cols, ~213 ns) mostly hides under the
preceding bf16 174 ns column stream, but fully serializes (~330 ns slot)
if it follows another DR matmul's 87 ns stream. W stays resident in
SBUF; x arrives as 3 quarter-slabs per chunk ([128, 4*cw]) plus a small
fp8 slab, ordered so consumption follows wire arrival. PSUM->SBUF drains
on DVE (ACT sits behind wire-backpressured prologue W dma_starts);
out-DMAs ride gpsimd's empty wire except the last chunk, which spreads
over all four queues (scalar/sync/vector/gpsimd) to shrink the
end-of-kernel DMA backlog. A short dummy-matmul warmup keeps the PE HAM
clock at 2.4 GHz through the ~8us runtime/DMA preamble without delaying
the first real matmul past x-chunk-0 arrival. Host transposes the
per-core [3072, t_out] outputs back.
"""

import numpy as np
import ml_dtypes

import concourse.bacc as bacc
import concourse.mybir as mybir
import concourse.tile as tile

NCORES = 8
HIDDEN = 2048
QKV_OUT = 3072
P = 128
KO = HIDDEN // P          # 16 contraction tiles
JT = QKV_OUT // P         # 24 qkv output tiles

BF16 = ml_dtypes.bfloat16
FP8 = ml_dtypes.float8_e4m3
NP8 = 2                   # fp8 DoubleRow pairs (k-tiles 0..2*NP8-1)
K8T = 2                   # k-tiles per DoubleRow matmul (fixed by HW)
KB0 = NP8 * K8T           # first bf16 k-tile (4)
NKB = KO - KB0            # bf16 k-tiles (12)
NQ = 3                    # bf16 x quarter-slabs per chunk (4 k-tiles each)
KQ = NKB // NQ            # k-tiles per quarter (4)
SW8 = 16.0                # fp8 scale: w*SW8, x/SW8 (keeps W out of subnormals)
N_WARM = 13               # dummy matmuls bridging the runtime preamble

_cache: dict = {}


def _build(n_c: int, cw: int, cw_last: int):
    """Bass module for one core: outT[3072, t_out] = W-stationary matmul."""
    nc = bacc.Bacc("TRN2", target_bir_lowering=False, debug=False)
    t_out = (n_c - 1) * cw + cw_last
    bf16 = mybir.dt.bfloat16
    f32 = mybir.dt.float32

    fp8 = mybir.dt.float8e4
    ts8 = -(-cw // 16) * 16  # fp8 slab t-stride (DoubleRow needs step%16==0)

    # x quarter-slabs: row block (c*NQ+q)*128 is [128, KQ*cw], covering
    # bf16 k-tiles KB0+KQ*q .. KB0+KQ*(q+1)-1
    xq = nc.dram_tensor("xq", [n_c * NQ * P, KQ * cw], bf16, kind="ExternalInput")
    # fp8 copies of k-tiles 0..KB0-1, DoubleRow layout [p, pair, ks, t]
    x8q = nc.dram_tensor("x8q", [n_c * P, NP8, K8T, ts8], fp8, kind="ExternalInput")
    w8q = nc.dram_tensor("w8q", [JT * P, NP8, K8T, P], fp8, kind="ExternalInput")
    # W tiles: row block j*128 is [128, NKB*128]; col slice (k-KB0)*128 is
    # the [hidden-k, qkv-j] stationary tile
    wq = nc.dram_tensor("wq", [JT * P, NKB * P], bf16, kind="ExternalInput")
    out = nc.dram_tensor("outT", [QKV_OUT, t_out], f32, kind="ExternalOutput")

    with tile.TileContext(nc) as tc:
        with (
            tc.tile_pool(name="wp", bufs=JT - 1) as wp,
            tc.tile_pool(name="w0p", bufs=NQ + 2) as w0p,
            tc.tile_pool(name="xp", bufs=3 * NQ) as xp,
            tc.tile_pool(name="x8p", bufs=3) as x8p,
            tc.tile_pool(name="w8p", bufs=JT) as w8p,
            tc.tile_pool(name="pp", bufs=8, space="PSUM") as pp,
            tc.tile_pool(name="op", bufs=10) as op,
            tc.tile_pool(name="opl", bufs=4) as opl,
        ):
            # DMA emission order is tuned so arrival follows consumption.
            # During the prologue the three queues (sync/scalar/gpsimd)
            # share the 16 DMA engines' aggregate bandwidth, so chunk 0's
            # pieces are spread over all three queues -- gpsimd's wire is
            # empty until the first out-DMAs fire.
            def load_x(c, engs):
                t8 = x8p.tile([P, NP8, K8T, ts8], fp8, name=f"x8_{c}", tag="x8")
                engs[0].dma_start(out=t8[:], in_=x8q[c * P:(c + 1) * P])
                qs = [t8]
                for q in range(NQ):
                    t = xp.tile([P, KQ * cw], bf16, name=f"x_{c}_{q}", tag="x")
                    r0 = (c * NQ + q) * P
                    engs[q % len(engs)].dma_start(out=t[:], in_=xq[r0:r0 + P, :])
                    qs.append(t)
                return qs



            # PE warmup: the first ~8us are runtime preamble + queue config
            # with no data on chip; dummy matmuls on zeroed scratch keep the
            # HAM activity window busy so the PE is at 2.4 GHz (not the cold
            # 1.2) when the real stream starts, without outlasting the
            # arrival of chunk 0.
            wu_w = w0p.tile([P, P], bf16, name="wu_w", tag="wu")
            wu_x = w0p.tile([P, 512], bf16, name="wu_x", tag="wu")
            nc.vector.memset(wu_w[:], 0.0)
            nc.vector.memset(wu_x[:], 0.0)
            for i in range(N_WARM):
                wu_ps = pp.tile([P, 512], f32, name=f"wu_ps_{i}", tag="ps")
                nc.tensor.matmul(wu_ps[:], wu_w[:], wu_x[:],
                                 start=True, stop=True)

            def load_w8(j, eng):
                t = w8p.tile([P, NP8, K8T, P], fp8, name=f"w8_{j}", tag="w8")
                eng.dma_start(out=t[:], in_=w8q[j * P:(j + 1) * P])
                return t

            # chunk 0 + j=0 weights, arrival-ordered across three queues:
            #   consumption: {q0,w0q0} -> {x8,w8_0} -> {q1,w0q1} ->
            #   {q2,w0q2} -> W j=1 -> W j=2 ...
            #   sync: q0, w0q2, W-odd | scalar: w0q0, w8_0, q1, W-even |
            #   gpsimd: x8, w0q1, q2 (its wire is empty until j0 drains)
            x8_0 = x8p.tile([P, NP8, K8T, ts8], fp8, name="x8_0", tag="x8")
            xq0 = [xp.tile([P, KQ * cw], bf16, name=f"x_0_{q}", tag="x")
                   for q in range(NQ)]
            w0q = [w0p.tile([P, KQ * P], bf16, name=f"w0_{q}", tag="w0")
                   for q in range(NQ)]

            def w0_load(q, eng):
                eng.dma_start(out=w0q[q][:],
                              in_=wq[0:P, q * KQ * P:(q + 1) * KQ * P])

            nc.sync.dma_start(out=xq0[0][:], in_=xq[0:P, :])
            w0_load(0, nc.scalar)
            w8t = [load_w8(0, nc.scalar)]
            nc.gpsimd.dma_start(out=x8_0[:], in_=x8q[0:P])
            nc.scalar.dma_start(out=xq0[1][:], in_=xq[P:2 * P, :])
            nc.gpsimd.dma_start(out=xq0[2][:], in_=xq[2 * P:3 * P, :])
            w0_load(1, nc.scalar)
            w0_load(2, nc.sync)

            xc = {0: [x8_0] + xq0}
            wt = [None]
            for j in range(1, JT):
                w = wp.tile([P, NKB * P], bf16, name=f"w_{j}", tag="w")
                eng = nc.sync if j % 2 == 1 else nc.scalar
                eng.dma_start(out=w[:], in_=wq[j * P:(j + 1) * P, :])
                w8t.append(load_w8(j, eng))
                wt.append(w)
            if n_c > 1:
                xc[1] = load_x(1, [nc.sync])

            def w_slice(j, k):
                kk = k - KB0
                if j == 0:
                    q, kq = divmod(kk, KQ)
                    return w0q[q][:, kq * P:(kq + 1) * P]
                return wt[j][:, kk * P:(kk + 1) * P]

            # k-loop order per (c, j): bf16 k=4 starts the PSUM group, the
            # two DoubleRow pairs sit mid-stream after a bf16 matmul (their
            # 256-col LDWEIGHTS hides under the preceding 174ns stream;
            # back-to-back DR matmuls would expose it), and consumption
            # follows quarter arrival order (q0, q1, q2).
            for c in range(n_c):
                xt = xc.pop(c)
                if c + 2 < n_c:
                    xc[c + 2] = load_x(c + 2, [nc.sync])
                cwc = cw if c < n_c - 1 else cw_last
                x8t = xt[0]

                def bf(ps, j, k, xt=xt, cwc=cwc, first=False, last=False):
                    q, kq = divmod(k - KB0, KQ)
                    nc.tensor.matmul(
                        ps[:], w_slice(j, k),
                        xt[1 + q][:, kq * cw:kq * cw + cwc],
                        start=first, stop=last)

                def dr(ps, j, pair, x8t=x8t, cwc=cwc):
                    nc.tensor.matmul(
                        ps[:], w8t[j][:, pair], x8t[:, pair, :, :cwc],
                        start=False, stop=False,
                        perf_mode=mybir.MatmulPerfMode.DoubleRow)

                for j in range(JT):
                    ps = pp.tile([P, cwc], f32, name=f"ps_{c}_{j}", tag="ps")
                    bf(ps, j, 4, first=True)
                    dr(ps, j, 0)
                    for k in (5, 6, 7, 8, 9):
                        bf(ps, j, k)
                    dr(ps, j, 1)
                    for k in (10, 11, 12, 13, 14):
                        bf(ps, j, k)
                    bf(ps, j, 15, last=True)
                    pool = op if cwc == cw else opl
                    ot = pool.tile([P, cwc], f32, name=f"o_{c}_{j}", tag="o")
                    # all copies on DVE: an ACT copy on the scalar queue sits
                    # behind wire-backpressured prologue W dma_starts (strict
                    # FIFO) and its PSUM bank can't free until the wire drains
                    nc.vector.tensor_copy(ot[:], ps[:])
                    # Out-transfers ride gpsimd's empty wire (the sync/scalar
                    # wires carry the W+x prologue backlog that would stall
                    # the 10-deep ot ring). The last chunk spreads over all
                    # four queues -- empty by then -- so the final 5MB
                    # backlog drains ~4x faster.
                    # Last chunk switches to sync/scalar -- empty by then --
                    # so gpsimd's slow postamble DRAIN fires early, off the
                    # critical path.
                    if c < n_c - 1:
                        eng = nc.gpsimd
                    else:
                        eng = nc.scalar if j % 2 == 0 else nc.sync
                    eng.dma_start(
                        out=out[j * P:(j + 1) * P, c * cw:c * cw + cwc],
                        in_=ot[:])
    nc.compile()
    return nc


def _plan(counts):
    """Allocate 8 cores to experts proportionally (largest remainder),
    then split each expert's token range into per-core contiguous spans.
    Returns (spans, t_max): spans[c] = (expert, start, length)."""
    total = int(counts.sum())
    ne = len(counts)
    active = [e for e in range(ne) if counts[e] > 0]
    quota = {e: counts[e] * NCORES / total for e in active}
    alloc = {e: max(1, int(quota[e])) for e in active}
    while sum(alloc.values()) > NCORES:  # too many mins; shrink largest
        shrinkable = [e for e in active if alloc[e] > 1]
        e = max(shrinkable, key=lambda e: alloc[e] - quota[e])
        alloc[e] -= 1
    rema = sorted(active, key=lambda e: quota[e] - alloc[e], reverse=True)
    i = 0
    while sum(alloc.values()) < NCORES:
        alloc[rema[i % len(rema)]] += 1
        i += 1
    spans = []
    starts = np.concatenate([[0], np.cumsum(counts)])
    for e in active:
        k = alloc[e]
        base, extra = divmod(int(counts[e]), k)
        off = int(starts[e])
        for j in range(k):
            ln = base + (1 if j < extra else 0)
            spans.append((e, off, ln))
            off += ln
    t_max = max(ln for _, _, ln in spans)
    return spans, t_max


def _chunking(t_max):
    n_c = max(1, -(-t_max // 512))
    cw = -(-t_max // n_c)
    cw = (cw + 1) // 2 * 2  # even token count -> 4B-aligned bf16 lines
    cw_last = t_max - (n_c - 1) * cw  # last chunk computes only real tokens
    return n_c, cw, cw_last


def _runner(n_c: int, cw: int, cw_last: int):
    """Compiled 8-core executor, cached so repeat kernel() calls skip jax
    retracing. Mirrors bass2jax.run_bass_via_pjrt's multi-core path
    (concat per-core inputs on axis 0 + shard_map)."""
    import jax
    import jax.numpy as jnp
    from jax.sharding import Mesh, PartitionSpec
    from jax.experimental.shard_map import shard_map
    from concourse import bass2jax, mybir as mb

    nc = _build(n_c, cw, cw_last)
    bass2jax.install_neuronx_cc_hook()

    part_name = nc.partition_id_tensor.name if nc.partition_id_tensor else None
    in_names, out_names, out_avals = [], [], []
    for alloc in nc.m.functions[0].allocations:
        if not isinstance(alloc, mb.MemoryLocationSet):
            continue
        name = alloc.memorylocations[0].name
        if alloc.kind == "ExternalInput":
            if name != part_name:
                in_names.append(name)
        elif alloc.kind == "ExternalOutput":
            out_names.append(name)
            out_avals.append(
                jax.core.ShapedArray(tuple(alloc.tensor_shape),
                                     mb.dt.np(alloc.dtype)))
    n_params = len(in_names)
    n_outs = len(out_names)
    bind_names = in_names + out_names + ([part_name] if part_name else [])

    def _body(*args):
        operands = list(args)
        if part_name:
            operands.append(bass2jax.partition_id_tensor())
        outs = bass2jax._bass_exec_p.bind(
            *operands,
            out_avals=tuple(out_avals),
            in_names=tuple(bind_names),
            out_names=tuple(out_names),
            lowering_input_output_aliases=(),
            sim_require_finite=True,
            sim_require_nnan=True,
            nc=nc,
        )
        return tuple(outs)

    devices = jax.devices()[:NCORES]
    mesh = Mesh(np.asarray(devices), ("core",))
    sharded = jax.jit(
        shard_map(_body, mesh=mesh,
                  in_specs=(PartitionSpec("core"),) * (n_params + n_outs),
                  out_specs=(PartitionSpec("core"),) * n_outs,
                  check_rep=False),
        donate_argnums=tuple(range(n_params, n_params + n_outs)),
        keep_unused=True,
    )

    def run(in_maps):
        concat_in = [
            np.concatenate([m[name] for m in in_maps], axis=0)
            for name in in_names
        ]
        zeros = [np.zeros((NCORES * a.shape[0], *a.shape[1:]), a.dtype)
                 for a in out_avals]
        outs = sharded(*concat_in, *zeros)
        return [
            {name: np.asarray(outs[i]).reshape(NCORES, *out_avals[i].shape)[c]
             for i, name in enumerate(out_names)}
            for c in range(NCORES)
        ]

    return run


def _prepare(x, W, mm):
    """Host-side plan + per-core input maps. Returns
    (n_c, cw, cw_last, spans, in_maps, perm)."""
    perm = None
    if np.any(np.diff(mm) < 0):  # insurance: tokens not pre-sorted
        perm = np.argsort(mm, kind="stable")
        x = x[perm]
        mm = mm[perm]

    E = W.shape[0]
    counts = np.bincount(mm.astype(np.int64), minlength=E)
    spans, t_max = _plan(counts)
    n_c, cw, cw_last = _chunking(t_max)
    t_pad = n_c * cw

    ts8 = -(-cw // 16) * 16
    kb_d = KB0 * P  # hidden dims covered by fp8 (512)

    # per-expert W repack:
    #   bf16 [j, p, kk*128+m] = W[e][j*128+m, (KB0+kk)*128+p]
    #   fp8  [j*128+p, pair, ks, m] = W[j*128+m, (2*pair+ks)*128+p] * SW8
    wqs = {}
    in_maps = []
    for e, off, ln in spans:
        if e not in wqs:
            wb = np.ascontiguousarray(
                W[e][:, kb_d:].reshape(JT, P, NKB, P).transpose(0, 3, 2, 1)
                .reshape(JT * P, NKB * P).astype(BF16))
            w8 = np.ascontiguousarray(
                (W[e][:, :kb_d] * SW8).reshape(JT, P, NP8, K8T, P)
                .transpose(0, 4, 2, 3, 1).reshape(JT * P, NP8, K8T, P)
                .astype(FP8))
            wqs[e] = (wb, w8)
        xs = np.zeros((t_pad, HIDDEN), dtype=np.float32)
        xs[:ln] = x[off:off + ln]
        # bf16 quarters: [c, t, kk, p] -> [c, q, p, kq, t]
        xr = (xs[:, kb_d:].reshape(n_c, cw, NKB, P).transpose(0, 2, 3, 1)
              .reshape(n_c, NQ, KQ, P, cw).transpose(0, 1, 3, 2, 4)
              .reshape(n_c * NQ * P, KQ * cw).astype(BF16))
        # fp8 DoubleRow x: [c*128+p, pair, ks, t]
        #   = x[c*cw+t, (2*pair+ks)*128+p] / SW8
        x8 = np.zeros((n_c, P, NP8, K8T, ts8), dtype=FP8)
        x8[:, :, :, :, :cw] = (xs[:, :kb_d] / SW8)\
            .reshape(n_c, cw, NP8, K8T, P).transpose(0, 4, 2, 3, 1)\
            .astype(FP8)
        in_maps.append({"xq": np.ascontiguousarray(xr),
                        "x8q": x8.reshape(n_c * P, NP8, K8T, ts8),
                        "wq": wqs[e][0], "w8q": wqs[e][1]})
    return n_c, cw, cw_last, spans, in_maps, perm


def profile_setup(inputs):
    """Build the compiled module + per-core input maps for test.py's
    profiled run (not used by the grading harness)."""
    x = np.ascontiguousarray(np.asarray(inputs["x"], dtype=np.float32))
    W = np.asarray(inputs["W"], dtype=np.float32)
    mm = np.asarray(inputs["modality_mapping"])
    n_c, cw, cw_last, spans, in_maps, perm = _prepare(x, W, mm)
    return _build(n_c, cw, cw_last), in_maps


def kernel(x, W, modality_mapping):
    x = np.ascontiguousarray(np.asarray(x, dtype=np.float32))
    W = np.asarray(W, dtype=np.float32)
    mm = np.asarray(modality_mapping)

    n_c, cw, cw_last, spans, in_maps, perm = _prepare(x, W, mm)

    key = (n_c, cw, cw_last)
    if key not in _cache:
        _cache[key] = _runner(*key)
    run = _cache[key]

    results = run(in_maps)

    T = x.shape[0]
    out = np.empty((T, QKV_OUT), dtype=np.float32)
    for c, (e, off, ln) in enumerate(spans):
        out[off:off + ln] = results[c]["outT"][:, :ln].T
    if perm is not None:
        inv = np.empty_like(perm)
        inv[perm] = np.arange(T)
        out = out[inv]
    return out
